# revision 51
# baseline (speedup 1.0000x reference)
"""Trainium2 Bass kernel for causal linear attention (elu+1 feature map) with
output projection + sigmoid gate residual mixing.

Reference computation (B=2, S=1024, D=512, H=8, hd=64):
    q = fmap(x@Wq), k = fmap(x@Wk), v = x@Wv          (fmap = elu+1)
    attn[s] = q[s] . cumsum_t<=s(k[t] v[t]^T) / (q[s] . cumsum(k) + 1e-6)
    out = attn@Wo + bo
    gate = sigmoid([x, out]@Wg + bg)
    y = x + gate*(out - x)

Sharding: 8 cores = (b in {0,1}) x (s-quarter j in {0..3}).  Core (b,j) owns
rows [256j, 256j+256) of batch b.  The causal prefix state (sum over earlier
rows of k^T [v|1]) is recomputed locally from a zero-padded prefix input
(uniform SPMD instruction stream; a mask column keeps padding out of the
state).  No cross-core communication.

Layouts: activations are kept feature-major ("fm", [d, s]) for contractions
over features and row-major for contractions over time + final I/O.  The host
supplies pre-transposed copies of x (layout prep during sharding), and
pre-rounds every tensor consumed by a float32r matmul (f32r = RNE to 11
mantissa bits, verified against the HW cast-DMA) so all loads are plain
HWDGE DMAs of the final bits.
"""

import os
import functools
import numpy as np

B, S, D = 2, 1024, 512
H, HD = 8, 64
SQ = 256          # rows owned per core
PRE = 3 * SQ      # padded prefix rows
NCORE = 8
P = 128

LAST_EXEC_NS = [None]


@functools.lru_cache(maxsize=1)
def _build():
    import concourse.bass as bass
    import concourse.mybir as mybir
    import concourse.tile as tile
    from concourse import bacc

    f32 = mybir.dt.float32
    f32r = mybir.dt.float32r

    nc = bacc.Bacc(
        "TRN2", target_bir_lowering=False, debug=False, num_devices=NCORE
    )

    dx_own = nc.dram_tensor("x_own", [SQ, D], f32, kind="ExternalInput").ap()
    dx_ownT = nc.dram_tensor("x_ownT", [D, SQ], f32r, kind="ExternalInput").ap()
    dx_preT = nc.dram_tensor("x_preT", [D, PRE], f32r, kind="ExternalInput").ap()
    dmask8 = nc.dram_tensor("mask8", [PRE, H], f32, kind="ExternalInput").ap()
    dwq = nc.dram_tensor("Wq", [D, D], f32r, kind="ExternalInput").ap()
    dwk = nc.dram_tensor("Wk", [D, D], f32r, kind="ExternalInput").ap()
    dwv = nc.dram_tensor("Wv", [D, D], f32r, kind="ExternalInput").ap()
    dwo = nc.dram_tensor("Wo", [D, D], f32r, kind="ExternalInput").ap()
    dbo = nc.dram_tensor("bo", [D], f32r, kind="ExternalInput").ap()
    dwg = nc.dram_tensor("Wg", [2 * D, D], f32r, kind="ExternalInput").ap()
    dbg = nc.dram_tensor("bg", [D], f32r, kind="ExternalInput").ap()
    dy = nc.dram_tensor("y", [SQ, D], f32, kind="ExternalOutput").ap()

    with tile.TileContext(nc) as tc:
        _emit(nc, tc, mybir, dx_own, dx_ownT, dx_preT, dmask8, dwq, dwk, dwv,
              dwo, dbo, dwg, dbg, dy)

    nc.compile()
    return nc


def _emit(nc, tc, mybir, dx_own, dx_ownT, dx_preT, dmask8, dwq, dwk, dwv, dwo,
          dbo, dwg, dbg, dy):
    f32 = mybir.dt.float32
    f32r = mybir.dt.float32r
    bf16 = mybir.dt.bfloat16
    AF = mybir.ActivationFunctionType
    OP = mybir.AluOpType
    NPRE = PRE // P           # 6 prefix chunks
    NCH = SQ // P             # 2 own chunks

    import contextlib
    import bass_rust as _br

    def chain(mms):
        # Accumulating matmuls into one PSUM bank must execute in emission
        # order (start=True first, stop=True last) — the Tile scheduler is
        # otherwise free to reorder same-engine instructions.
        for later, earlier in zip(mms[1:], mms[:-1]):
            _br.add_dep_helper(later.ins, earlier.ins, sync=False,
                               reason="psum accumulation order")

    ctx = contextlib.ExitStack()
    with ctx:
        consts = ctx.enter_context(tc.tile_pool(name="consts", bufs=1))
        fmtmp = ctx.enter_context(tc.tile_pool(name="fmtmp", bufs=4))
        prework = ctx.enter_context(tc.tile_pool(name="prework", bufs=4))
        attnwork = ctx.enter_context(tc.tile_pool(name="attnwork", bufs=4))
        outwork = ctx.enter_context(tc.tile_pool(name="outwork", bufs=3))
        # PSUM pools: total concurrent slots must stay <= 8 banks
        pp = ctx.enter_context(tc.tile_pool(name="pp", bufs=3, space="PSUM"))
        pA = ctx.enter_context(tc.tile_pool(name="pA", bufs=3, space="PSUM"))
        pn = ctx.enter_context(tc.tile_pool(name="pn", bufs=2, space="PSUM"))

        # ---------------- constant / persistent loads ----------------
        # Split into per-d-tile / per-chunk DMAs with separate tiles so the
        # prefix pipeline starts as soon as its first operands land, and the
        # transfers spread over multiple DMA queues.
        # DMA emission order == first-use order so the (mostly serial) input
        # stream feeds the compute pipeline just in time.
        dwkr = dwk.rearrange("(t p) e -> p t e", p=P)
        dwvr = dwv.rearrange("(t p) e -> p t e", p=P)
        dwqr = dwq.rearrange("(t p) e -> p t e", p=P)
        dwgr = dwg.rearrange("(t p) e -> p t e", p=P)
        dxpr = dx_preT.rearrange("(t p) s -> p t s", p=P)

        xT_pre_t = [consts.tile([P, 4, P], f32r, tag=f"xpre{c}",
                                name=f"xpre{c}") for c in range(NPRE)]
        nc.sync.dma_start(out=xT_pre_t[0], in_=dxpr[:, :, 0:P])
        wk_t, wv_t, wq_t = [], [], []
        for dt in range(4):
            t = consts.tile([P, D], f32r, tag=f"wk{dt}")
            nc.sync.dma_start(out=t, in_=dwkr[:, dt, :])
            wk_t.append(t)
        for dt in range(4):
            t = consts.tile([P, D], f32r, tag=f"wv{dt}")
            nc.sync.dma_start(out=t, in_=dwvr[:, dt, :])
            wv_t.append(t)
        for c in range(1, NPRE):
            nc.sync.dma_start(out=xT_pre_t[c], in_=dxpr[:, :, P * c:P * c + P])
        m8 = consts.tile([P, NPRE, H], f32)
        nc.sync.dma_start(out=m8,
                          in_=dmask8.rearrange("(c p) h -> p c h", p=P))
        xT_own = consts.tile([P, 4, SQ], f32r)
        nc.sync.dma_start(out=xT_own,
                          in_=dx_ownT.rearrange("(t p) s -> p t s", p=P))
        for dt in range(4):
            t = consts.tile([P, D], f32r, tag=f"wq{dt}")
            nc.sync.dma_start(out=t, in_=dwqr[:, dt, :])
            wq_t.append(t)
        # Wo in [d_local, head, e] layout so per-head K=64 contractions line up
        # with base-0 attn tiles
        wo64 = consts.tile([HD, H, D], f32r)
        nc.sync.dma_start(out=wo64, in_=dwo.rearrange("(h d) e -> d h e", d=HD))
        wg_t = []
        for kt in range(8):
            t = consts.tile([P, D], f32r, tag=f"wg{kt}")
            nc.sync.dma_start(out=t, in_=dwgr[:, kt, :])
            wg_t.append(t)
        bo_row = consts.tile([1, D], f32r)
        nc.sync.dma_start(out=bo_row, in_=dbo.rearrange("(o e) -> o e", o=1))
        bg_row = consts.tile([1, D], f32r)
        nc.sync.dma_start(out=bg_row, in_=dbg.rearrange("(o e) -> o e", o=1))
        bo_fm = consts.tile([P, 4], f32r)
        nc.sync.dma_start(out=bo_fm, in_=dbo.rearrange("(t p) -> p t", p=P))

        ones1_f = consts.tile([1, P], f32)
        nc.vector.memset(ones1_f, 1.0)
        ones1 = consts.tile([1, P], f32r)
        nc.vector.tensor_copy(out=ones1, in_=ones1_f)
        onesP_f = consts.tile([P, NCH * H], f32)
        nc.vector.memset(onesP_f, 1.0)

        # combined causal masks: cols 0:256 = [t <= s] for the t0 block,
        # cols 256:512 = [t+128 <= s] for the t1 block
        maskc = consts.tile([P, 2 * SQ], f32)
        nc.gpsimd.memset(maskc, 0.0)
        nc.gpsimd.affine_select(
            out=maskc[:, 0:SQ], in_=maskc[:, 0:SQ], compare_op=OP.is_gt,
            fill=1.0, base=0, pattern=[[-1, SQ]], channel_multiplier=1)
        nc.gpsimd.affine_select(
            out=maskc[:, SQ:], in_=maskc[:, SQ:], compare_op=OP.is_gt,
            fill=1.0, base=P, pattern=[[-1, SQ]], channel_multiplier=1)

        x_rm = consts.tile([P, NCH, D], f32)
        nc.sync.dma_start(out=x_rm, in_=dx_own.rearrange("(c p) e -> p c e", p=P))

        def fmap_from_psum(psum_ap, out_ap, width, pool, eng=None):
            """out = min(exp(t),1) + relu(t) elementwise from a PSUM tile."""
            e_t = pool.tile([P, width], f32, tag="fm_e")
            nc.scalar.activation(out=e_t, in_=psum_ap, func=AF.Exp)
            r_t = pool.tile([P, width], f32, tag="fm_r")
            nc.scalar.activation(out=r_t, in_=psum_ap, func=AF.Relu)
            (eng or nc.vector).scalar_tensor_tensor(
                out=out_ap, in0=e_t, scalar=1.0, in1=r_t,
                op0=OP.min, op1=OP.add)

        # ---------------- prefix state ----------------
        # state[64*(h%2):+64, h//2, :] accumulates K_h^T [V_h | mask] over all
        # prefix chunks.
        # full-bank shape (512 f32/partition) so partition-offset matmul
        # output slices stay inside one PSUM bank
        state_psum = pn.tile([P, 4, P], f32, tag="pn")
        state_mms = []
        k_rm_t, v_pre_t = {}, {}

        def emit_prefix_proj(c):
            ps_k = pp.tile([P, D], f32, tag="pp")
            chain([nc.tensor.matmul(
                ps_k, lhsT=xT_pre_t[c][:, dt, :],
                rhs=wk_t[dt],
                start=(dt == 0), stop=(dt == 3)) for dt in range(4)])
            k_rm = prework.tile([P, D], bf16, tag="k_rm")
            fmap_from_psum(ps_k, k_rm, D, prework)
            k_rm_t[c] = k_rm

            ps_v = pp.tile([P, D], f32, tag="pp")
            chain([nc.tensor.matmul(
                ps_v, lhsT=xT_pre_t[c][:, dt, :],
                rhs=wv_t[dt],
                start=(dt == 0), stop=(dt == 3)) for dt in range(4)])
            v_pre = prework.tile([P, H, HD + 1], bf16, tag="v_pre")
            nc.vector.tensor_copy(
                out=v_pre[:, :, 0:HD],
                in_=ps_v.rearrange("p (h e) -> p h e", h=H))
            nc.vector.tensor_copy(
                out=v_pre[:, :, HD:HD + 1],
                in_=m8[:, c, :].rearrange("p (h o) -> p h o", o=1))
            v_pre_t[c] = v_pre

        def emit_state(c):
            k_rm, v_pre = k_rm_t.pop(c), v_pre_t.pop(c)
            for h in range(H):
                r, p2 = h % 2, h // 2
                # one accumulation group per 64-partition half of the bank:
                # the start's pending-zero covers only the partitions it
                # touches, so each r-group needs its own start/stop
                state_mms.append(nc.tensor.matmul(
                    state_psum[64 * r:64 * r + 64, p2, 0:HD + 1],
                    lhsT=k_rm[:, HD * h:HD * h + HD],
                    rhs=v_pre[:, h, :],
                    start=(c == 0 and h == r),
                    stop=(c == NPRE - 1 and h == H - 2 + r),
                    tile_position=(0, 64 * r),
                    skip_group_check=True))

        emit_prefix_proj(0)
        for c in range(1, NPRE):
            emit_prefix_proj(c)
            emit_state(c - 1)
        emit_state(NPRE - 1)
        chain(state_mms)

        state_sb = consts.tile([P, 4, HD + 1], f32r)
        nc.vector.tensor_copy(out=state_sb, in_=state_psum[:, :, 0:HD + 1])

        # ---------------- own projections ----------------
        q_fm = consts.tile([P, 4, SQ], f32r)
        k_fm = consts.tile([P, 4, SQ], f32r)
        for (w_t, dst) in ((wq_t, q_fm), (wk_t, k_fm)):
            for et in range(4):
                ps = pp.tile([P, SQ], f32, tag="pp")
                chain([nc.tensor.matmul(
                    ps, lhsT=w_t[dt][:, P * et:P * et + P],
                    rhs=xT_own[:, dt, :],
                    start=(dt == 0), stop=(dt == 3)) for dt in range(4)])
                fmap_from_psum(ps, dst[:, et, :], SQ, fmtmp)

        v_own = consts.tile([P, NCH, H, HD + 1], f32r)
        nc.vector.tensor_copy(
            out=v_own[:, :, :, HD:HD + 1],
            in_=onesP_f.rearrange("p (c h o) -> p c h o", c=NCH, h=H))
        for c2 in range(NCH):
            ps = pp.tile([P, D], f32, tag="pp")
            chain([nc.tensor.matmul(
                ps, lhsT=xT_own[:, dt, P * c2:P * c2 + P],
                rhs=wv_t[dt],
                start=(dt == 0), stop=(dt == 3)) for dt in range(4)])
            nc.vector.tensor_copy(
                out=v_own[:, c2, :, 0:HD],
                in_=ps.rearrange("p (h e) -> p h e", h=H))

        # ---------------- attention (one 256-row block, t-subblocks of 128) --
        attn_all = consts.tile([HD, H, SQ], f32r)
        for h in range(H):
            r, p2 = h % 2, h // 2
            qh = q_fm[64 * r:64 * r + 64, p2, :]
            kh = k_fm[64 * r:64 * r + 64, p2, :]

            a01 = pA.tile([P, 2 * SQ], f32, tag="pA")
            chain([
                nc.tensor.matmul(a01[:, 0:SQ], lhsT=kh[:, 0:P],
                                 rhs=qh, start=True, stop=False),
                nc.tensor.matmul(a01[:, SQ:], lhsT=kh[:, P:SQ],
                                 rhs=qh, start=False, stop=True),
            ])
            amc = attnwork.tile([P, 2 * SQ], f32r, tag="amc")
            nc.vector.tensor_mul(amc, a01, maskc)
            am0 = amc[:, 0:SQ]
            am1 = amc[:, SQ:]

            numt = pn.tile([HD + 1, SQ], f32, tag="pn")
            chain([
                nc.tensor.matmul(numt, lhsT=v_own[:, 0, h, :],
                                 rhs=am0, start=True,
                                 stop=False),
                nc.tensor.matmul(numt, lhsT=v_own[:, 1, h, :],
                                 rhs=am1, start=False,
                                 stop=False),
                nc.tensor.matmul(numt,
                                 lhsT=state_sb[64 * r:64 * r + 64, p2, :]
                                 ,
                                 rhs=qh, start=False,
                                 stop=True),
            ])

            rec = attnwork.tile([P, SQ], f32, tag="rec")
            nc.vector.reciprocal(out=rec[64:65, :], in_=numt[HD:HD + 1, :])
            # partition_broadcast only works from partition 0; gpsimd
            # tensor_copy shifts partitions (DVE/ACT cannot)
            rec0 = attnwork.tile([1, SQ], f32, tag="rec0")
            nc.gpsimd.tensor_copy(out=rec0, in_=rec[64:65, :])
            recb = attnwork.tile([HD, SQ], f32, tag="recb")
            nc.gpsimd.partition_broadcast(recb, rec0)
            nc.vector.tensor_mul(attn_all[:, h, :], numt[0:HD, :], recb)

        # ---------------- output projection (fm, for the gate matmul) -------
        outT = consts.tile([P, 4, SQ], f32r)
        for et in range(4):
            ps = pp.tile([P, SQ], f32, tag="pp")
            chain([nc.tensor.matmul(
                ps, lhsT=wo64[:, h, P * et:P * et + P],
                rhs=attn_all[:, h, :],
                start=(h == 0), stop=(h == H - 1)) for h in range(H)])
            nc.vector.tensor_scalar_add(
                out=outT[:, et, :], in0=ps,
                scalar1=bo_fm[:, et:et + 1].bitcast(f32))

        # ---------------- out (row-major) + gate + final mix per chunk ------
        for c2 in range(NCH):
            ps_o = pp.tile([P, D], f32, tag="pp")
            o_mms = [nc.tensor.matmul(
                ps_o, lhsT=attn_all[:, h, P * c2:P * c2 + P],
                rhs=wo64[:, h, :],
                start=(h == 0), stop=False) for h in range(H)]
            o_mms.append(nc.tensor.matmul(ps_o, lhsT=ones1, rhs=bo_row,
                                          start=False, stop=True))
            chain(o_mms)

            # d1 = out - x needs no gate: emitted before the gate matmuls so
            # it overlaps them instead of serializing after the sigmoid
            d1 = outwork.tile([P, D], f32, tag="d1")
            nc.vector.tensor_sub(d1, ps_o, x_rm[:, c2, :])

            # gate in two 256-column halves: the sigmoid + final mix of one
            # half overlaps the other half's matmuls, and the two stores go
            # out on separate HWDGE rings (sync vs scalar)
            y_sb = outwork.tile([P, D], f32, tag="ysb")
            for half in range(2):
                sl = slice(256 * half, 256 * half + 256)
                ps_g = pp.tile([P, SQ], f32, tag="pp")
                g_mms = [nc.tensor.matmul(
                    ps_g, lhsT=xT_own[:, dt, P * c2:P * c2 + P],
                    rhs=wg_t[dt][:, sl],
                    start=(dt == 0), stop=False) for dt in range(4)]
                g_mms += [nc.tensor.matmul(
                    ps_g, lhsT=outT[:, ft, P * c2:P * c2 + P],
                    rhs=wg_t[4 + ft][:, sl],
                    start=False, stop=False) for ft in range(4)]
                g_mms.append(nc.tensor.matmul(
                    ps_g, lhsT=ones1, rhs=bg_row[:, sl],
                    start=False, stop=True))
                chain(g_mms)

                gate_sb = outwork.tile([P, SQ], f32, tag=f"gate{half}")
                nc.scalar.activation(out=gate_sb, in_=ps_g, func=AF.Sigmoid)
                d2 = outwork.tile([P, SQ], f32, tag=f"d2{half}")
                eng = nc.gpsimd if half == 0 else nc.vector
                eng.tensor_mul(d2, gate_sb, d1[:, sl])
                nc.vector.tensor_add(y_sb[:, sl], x_rm[:, c2, sl], d2)
                deng = nc.sync if half == 0 else nc.scalar
                deng.dma_start(
                    out=dy.rearrange("(c p) e -> p c e", p=P)[:, c2, sl],
                    in_=y_sb[:, sl])


def _round_f32r(x):
    # float32r = RNE to 11 mantissa bits (verified against HW cast-DMA)
    xi = x.view(np.uint32).astype(np.uint64)
    bias = ((xi >> 12) & 1) + (1 << 11) - 1
    return ((((xi + bias) >> 12) << 12) & 0xFFFFFFFF).astype(np.uint32).view(np.float32)


def _shard_inputs(inputs):
    x = np.ascontiguousarray(np.asarray(inputs["x"], dtype=np.float32))
    shared = {}
    for name in ("Wq", "Wk", "Wv", "Wo", "bo", "Wg", "bg"):
        shared[name] = _round_f32r(np.ascontiguousarray(
            np.asarray(inputs[name], dtype=np.float32)))
    in_maps = []
    for c in range(NCORE):
        b, j = c // 4, c % 4
        r0 = SQ * j
        x_own = x[b, r0:r0 + SQ]
        x_preT = np.zeros((D, PRE), np.float32)
        x_preT[:, :r0] = x[b, :r0].T
        mask8 = np.zeros((PRE, H), np.float32)
        mask8[:r0] = 1.0
        m = {"x_own": np.ascontiguousarray(x_own),
             "x_ownT": _round_f32r(np.ascontiguousarray(x_own.T)),
             "x_preT": _round_f32r(x_preT), "mask8": mask8}
        m.update(shared)
        in_maps.append(m)
    return in_maps


def kernel(**inputs):
    from concourse import bass_utils

    nc = _build()
    in_maps = _shard_inputs(inputs)
    trace = os.environ.get("BASS_KERNEL_TRACE", "0") == "1"
    res = bass_utils.run_bass_kernel_spmd(
        nc, in_maps, core_ids=list(range(NCORE)), trace=trace)
    LAST_EXEC_NS[0] = res.exec_time_ns
    x = np.asarray(inputs["x"], dtype=np.float32)
    y = np.empty_like(x)
    for c in range(NCORE):
        b, j = c // 4, c % 4
        y[b, SQ * j:SQ * j + SQ] = res.results[c]["y"]
    return y
